# revision 11
# baseline (speedup 1.0000x reference)
"""MoE layer (top-2 of 8 experts) on 8 Trainium2 NeuronCores.

Strategy (expert parallelism, host-side dispatch):
  - Router (x @ Wr -> softmax -> top-k) is computed on host: it is ~0.05% of
    the total FLOPs.  Decisions use float64 so near-ties resolve exactly.
  - Tokens are gathered per expert on host ("all-to-all dispatch"), padded to
    a common per-core count T_CORE, and each core runs its expert's FFN:
        hT = relu(W1e.T @ xT + b1e);  yT = W2e.T @ hT + b2e
    in bf16 on the tensor engine (fp32 PSUM accumulation).
  - Host applies the top-k combine weights and scatter-adds back ("combine").

Per-core device layout (everything 128-partition tiled):
  xt  [8,128,T]   bf16  x gathered+transposed, D on partitions (8 k-tiles)
  w1  [32,128,8,128] bf16  W1e chunk [f,p,k,:] = W1e[k*128+p, f*128:(f+1)*128]
  w2  [8,128,32,128] bf16  W2e chunk [d,p,f,:] = W2e[f*128+p, d*128:(d+1)*128]
  b1  [128,32] f32 (per-partition bias per f-chunk), b2 [128,8] f32
  yt  [8,128,T]   f32   output, D on partitions
"""

import numpy as np
import ml_dtypes

import concourse.bass as bass
import concourse.mybir as mybir
import concourse.tile as tile
from concourse import bacc
from concourse.bass_utils import run_bass_kernel_spmd

BF16 = mybir.dt.bfloat16
F32 = mybir.dt.float32

N_CORES = 8
P = 128

# Populated by kernel() with the BassKernelResults of the device run so a
# test harness can read exec_time_ns when tracing is enabled (BASS_TRACE=1).
LAST_RESULTS = None


def _build_moe_ffn(T_CORE: int, groups: list[tuple[int, int]], D: int, F: int):
    """One expert's FFN over T_CORE tokens: yT = W2.T @ relu(W1.T @ xT + b1) + b2."""
    KD = D // P   # k-tiles over D (contraction of layer 1)
    KF = F // P   # f-chunks over F (rows of hT / contraction of layer 2)
    ND = D // P   # d-chunks of the output

    nc = bacc.Bacc("TRN2", target_bir_lowering=False, debug=False,
                   num_devices=N_CORES)
    xt_d = nc.dram_tensor("xt", [KD, P, T_CORE], BF16, kind="ExternalInput")
    w1_d = nc.dram_tensor("w1", [KF, P, KD, P], BF16, kind="ExternalInput")
    w2_d = nc.dram_tensor("w2", [ND, P, KF, P], BF16, kind="ExternalInput")
    b1_d = nc.dram_tensor("b1", [P, KF], F32, kind="ExternalInput")
    b2_d = nc.dram_tensor("b2", [P, ND], F32, kind="ExternalInput")
    yt_d = nc.dram_tensor("yt", [ND, P, T_CORE], F32, kind="ExternalOutput")

    with tile.TileContext(nc) as tc:
        with (
            tc.tile_pool(name="resident", bufs=1) as rpool,
            tc.tile_pool(name="w2s", bufs=2) as w2pool,
            tc.tile_pool(name="yout", bufs=3) as ypool,
            tc.tile_pool(name="ph", bufs=3, space="PSUM") as php,
            tc.tile_pool(name="py", bufs=3, space="PSUM") as pyp,
        ):
            # Activations: per-(group, k) DMAs in group order so group 0's
            # matmuls start after ~0.5 MB of DMA, not the full load.
            xt_sb = rpool.tile([P, KD, T_CORE], BF16, tag="xt")
            for (g0, gn) in groups:
                for k in range(KD):
                    nc.sync.dma_start(xt_sb[:, k, g0:g0 + gn],
                                      xt_d[k][:, g0:g0 + gn])
            b1_sb = rpool.tile([P, KF], F32, tag="b1")
            nc.sync.dma_start(b1_sb[:], b1_d[:])
            b2_sb = rpool.tile([P, ND], F32, tag="b2")
            nc.sync.dma_start(b2_sb[:], b2_d[:])
            # W1 resident (64 KB/partition), streamed per-f on the scalar
            # (Activation) HWDGE queue so it never queues behind xt.
            w1_sb = rpool.tile([P, KF, KD, P], BF16, tag="w1")
            for f in range(KF):
                nc.scalar.dma_start(w1_sb[:, f], w1_d[f])
            h_sb = rpool.tile([P, KF, T_CORE], BF16, tag="h")

            # layer 1: hT[f] = relu(sum_k W1[k,f].T @ xT[k] + b1[f])
            # Group-outer: all PE work for group g depends only on group g's
            # activation slices (+ the W1 stream), so the PE ramps up while
            # the rest of xt is still in flight.
            for (g0, gn) in groups:
                for f in range(KF):
                    ph = php.tile([P, 512], F32, tag="ph", name="ph")[:, :gn]
                    for k in range(KD):
                        nc.tensor.matmul(
                            ph, w1_sb[:, f, k, :], xt_sb[:, k, g0:g0 + gn],
                            start=(k == 0), stop=(k == KD - 1),
                        )
                    nc.scalar.activation(
                        h_sb[:, f, g0:g0 + gn], ph,
                        mybir.ActivationFunctionType.Relu,
                        bias=b1_sb[:, f:f + 1],
                    )

            # layer 2: yT[d] = sum_f W2[f,d].T @ hT[f] + b2[d]
            for d in range(ND):
                w2d = w2pool.tile([P, KF, P], BF16, tag="w2d")
                nc.scalar.dma_start(w2d[:], w2_d[d])
                for (g0, gn) in groups:
                    py = pyp.tile([P, 512], F32, tag="py", name="py")[:, :gn]
                    for f in range(KF):
                        nc.tensor.matmul(
                            py, w2d[:, f, :], h_sb[:, f, g0:g0 + gn],
                            start=(f == 0), stop=(f == KF - 1),
                        )
                    ysb = ypool.tile([P, 512], F32, tag="ysb", name="ysb")[:, :gn]
                    nc.vector.tensor_scalar_add(ysb, py, b2_sb[:, d:d + 1])
                    nc.sync.dma_start(yt_d[d][:, g0:g0 + gn], ysb)

    nc.compile()
    return nc


def _plan_groups(T_CORE: int) -> list[int]:
    """Split T_CORE (multiple of 128) into matmul token-group sizes <= 512.
    A small leading group lets the PE start early; remainder sits at the end."""
    if T_CORE <= 128:
        return [T_CORE]
    sizes = [128]
    mid = T_CORE - 128
    while mid > 0:
        gn = min(512, mid)
        sizes.append(gn)
        mid -= gn
    return sizes


def kernel(x, Wr, br, W1, b1, W2, b2, top_k):
    x = np.asarray(x, dtype=np.float32)
    Wr = np.asarray(Wr, dtype=np.float32)
    br = np.asarray(br, dtype=np.float32)
    W1 = np.asarray(W1, dtype=np.float32)
    b1 = np.asarray(b1, dtype=np.float32)
    W2 = np.asarray(W2, dtype=np.float32)
    b2 = np.asarray(b2, dtype=np.float32)
    K = int(np.asarray(top_k))

    B, S, D = x.shape
    E = Wr.shape[1]
    F = W1.shape[2]
    T = B * S
    xt = x.reshape(T, D)

    # --- host router (replicated): f32 probs to match the reference, f64 top-k
    logits = xt @ Wr + br
    lmax = logits.max(axis=1, keepdims=True)
    pexp = np.exp(logits - lmax)
    probs = pexp / pexp.sum(axis=1, keepdims=True)          # [T, E] f32
    logits64 = xt.astype(np.float64) @ Wr.astype(np.float64) + br
    # top-k by descending prob, ties -> lower index (jax.lax.top_k semantics)
    topi = np.argsort(-logits64, axis=1, kind="stable")[:, :K]  # [T, K]

    # --- dispatch: token lists per expert
    tok_idx = [np.where((topi == e).any(axis=1))[0] for e in range(E)]
    counts = np.array([len(ix) for ix in tok_idx])
    T_CORE = max(P, int(np.ceil(counts.max() / P)) * P)
    # Group order matters: a small FIRST group lets the PE start after only
    # ~0.5 MB of input DMA; a small LAST group shortens the evacuation tail.
    groups = []
    off = 0
    for gn in _plan_groups(T_CORE):
        groups.append((off, gn))
        off += gn

    in_maps = []
    for e in range(E):
        ix = tok_idx[e]
        xe = np.zeros((T_CORE, D), dtype=np.float32)
        xe[: len(ix)] = xt[ix]
        xte = np.ascontiguousarray(xe.T).astype(ml_dtypes.bfloat16)
        w1e = np.ascontiguousarray(
            W1[e].reshape(D // P, P, F // P, P).transpose(2, 1, 0, 3)
        ).astype(ml_dtypes.bfloat16)
        w2e = np.ascontiguousarray(
            W2[e].reshape(F // P, P, D // P, P).transpose(2, 1, 0, 3)
        ).astype(ml_dtypes.bfloat16)
        b1e = np.ascontiguousarray(b1[e].reshape(F // P, P).T)
        b2e = np.ascontiguousarray(b2[e].reshape(D // P, P).T)
        in_maps.append({
            "xt": xte.reshape(D // P, P, T_CORE),
            "w1": w1e,
            "w2": w2e,
            "b1": b1e,
            "b2": b2e,
        })

    nc = _build_moe_ffn(T_CORE, groups, D, F)
    res = run_bass_kernel_spmd(nc, in_maps, core_ids=list(range(N_CORES)))
    global LAST_RESULTS
    LAST_RESULTS = res

    # --- combine: out[t] += probs[t, e] * y_e[slot(t)]
    out = np.zeros((T, D), dtype=np.float32)
    for e in range(E):
        ix = tok_idx[e]
        yte = res.results[e]["yt"]                     # [D//P, P, T_CORE]
        ye = yte.reshape(D, T_CORE)[:, : len(ix)].T    # [c_e, D] token rows
        out[ix] += probs[ix, e][:, None] * ye

    return out.reshape(B, S, D), probs.reshape(B, S, E)


# revision 17
# speedup vs baseline: 1.1614x; 1.1614x over previous
"""MoE layer (top-2 of 8 experts) on 8 Trainium2 NeuronCores.

Strategy (expert parallelism, host-side dispatch):
  - Router (x @ Wr -> softmax -> top-k) is computed on host: it is ~0.05% of
    the total FLOPs.  Decisions use float64 so near-ties resolve exactly.
  - Tokens are gathered per expert on host ("all-to-all dispatch"), padded to
    a common per-core count T_CORE, and each core runs its expert's FFN:
        hT = relu(W1e.T @ xT + b1e);  yT = W2e.T @ hT + b2e
    in bf16 on the tensor engine (fp32 PSUM accumulation).
  - Host applies the top-k combine weights and scatter-adds back ("combine").

Per-core device layout (everything 128-partition tiled):
  xt  [8,128,T]   bf16  x gathered+transposed, D on partitions (8 k-tiles)
  w1  [32,128,8,128] bf16  W1e chunk [f,p,k,:] = W1e[k*128+p, f*128:(f+1)*128]
  w2  [8,128,32,128] bf16  W2e chunk [d,p,f,:] = W2e[f*128+p, d*128:(d+1)*128]
  b1  [128,32] f32 (per-partition bias per f-chunk), b2 [128,8] f32
  yt  [8,128,T]   f32   output, D on partitions
"""

import numpy as np
import ml_dtypes

import concourse.mybir as mybir
import concourse.tile as tile
from concourse import bacc
from concourse.bass_utils import run_bass_kernel_spmd

BF16 = mybir.dt.bfloat16
F32 = mybir.dt.float32

N_CORES = 8
P = 128

# Populated by kernel() with the BassKernelResults of the device run so a
# test harness can read exec_time_ns when tracing is enabled (BASS_TRACE=1).
LAST_RESULTS = None


def _build_moe_ffn(T_CORE: int, groups: list[tuple[int, int]], D: int, F: int):
    """One expert's FFN over T_CORE tokens: yT = W2.T @ relu(W1.T @ xT + b1) + b2."""
    KD = D // P   # k-tiles over D (contraction of layer 1)
    KF = F // P   # f-chunks over F (rows of hT / contraction of layer 2)
    ND = D // P   # d-chunks of the output

    nc = bacc.Bacc("TRN2", target_bir_lowering=False, debug=False,
                   num_devices=N_CORES)
    xt_d = nc.dram_tensor("xt", [KD, P, T_CORE], BF16, kind="ExternalInput")
    w1_d = nc.dram_tensor("w1", [KF, P, KD, P], BF16, kind="ExternalInput")
    w2_d = nc.dram_tensor("w2", [ND, P, KF, P], BF16, kind="ExternalInput")
    b1_d = nc.dram_tensor("b1", [P, KF], F32, kind="ExternalInput")
    b2_d = nc.dram_tensor("b2", [P, ND], F32, kind="ExternalInput")
    yt_d = nc.dram_tensor("yt", [ND, P, T_CORE], F32, kind="ExternalOutput")

    # DMA placement notes: nc.sync/nc.scalar dma_start lowers to DMA_DIRECT2D
    # which OCCUPIES the issuing engine for the transfer duration, and each
    # engine's queue is FIFO.  So: activations + outputs ride the (otherwise
    # idle) sync queue; W1/W2 ride the scalar queue, with W1 DMAs interleaved
    # behind layer-1 ACTIVATEs so the weight stream is self-clocked by compute
    # progress and never blocks PSUM evacuation.
    W1_AHEAD = 10  # W1 f-chunks prefetched before the first matmul

    with tile.TileContext(nc) as tc:
        with (
            tc.tile_pool(name="resident", bufs=1) as rpool,
            tc.tile_pool(name="w2s", bufs=3) as w2pool,
            tc.tile_pool(name="yout", bufs=3) as ypool,
            tc.tile_pool(name="ph", bufs=4, space="PSUM") as php,
            tc.tile_pool(name="py", bufs=4, space="PSUM") as pyp,
        ):
            # sync queue: activations in group order (group 0's matmuls
            # only wait on group 0's slices).  Biases ride the scalar queue.
            b1_sb = rpool.tile([P, KF], F32, tag="b1")
            nc.scalar.dma_start(b1_sb[:], b1_d[:])
            b2_sb = rpool.tile([P, ND], F32, tag="b2")
            nc.scalar.dma_start(b2_sb[:], b2_d[:])
            xt_sb = rpool.tile([P, KD, T_CORE], BF16, tag="xt")
            for (g0, gn) in groups:
                for k in range(KD):
                    nc.sync.dma_start(xt_sb[:, k, g0:g0 + gn],
                                      xt_d[k][:, g0:g0 + gn])
            # scalar queue: initial W1 window; the rest is emitted inside the
            # first group's f-loop.
            w1_sb = rpool.tile([P, KF, KD, P], BF16, tag="w1")
            for f in range(min(W1_AHEAD, KF)):
                nc.scalar.dma_start(w1_sb[:, f], w1_d[f])
            h_sb = rpool.tile([P, KF, T_CORE], BF16, tag="h")

            # layer 1: hT[f] = relu(sum_k W1[k,f].T @ xT[k] + b1[f])
            # Group-outer: group g's PE work depends only on group g's
            # activation slices plus the self-clocked W1 stream.
            for gi, (g0, gn) in enumerate(groups):
                for f in range(KF):
                    ph = php.tile([P, 512], F32, tag="ph", name="ph")[:, :gn]
                    for k in range(KD):
                        nc.tensor.matmul(
                            ph, w1_sb[:, f, k, :], xt_sb[:, k, g0:g0 + gn],
                            start=(k == 0), stop=(k == KD - 1),
                        )
                    nc.scalar.activation(
                        h_sb[:, f, g0:g0 + gn], ph,
                        mybir.ActivationFunctionType.Relu,
                        bias=b1_sb[:, f:f + 1],
                    )
                    if gi == 0 and f + W1_AHEAD < KF:
                        nc.scalar.dma_start(w1_sb[:, f + W1_AHEAD],
                                            w1_d[f + W1_AHEAD])

            # W2 stream (scalar queue, chunked so layer 2's first matmuls only
            # wait on the first quarter of W2[0]).  With bufs=3 the 4th DMA
            # blocks the scalar queue until layer 2 frees a slot — harmless,
            # the scalar engine has no further work.
            W2_CHUNK = max(1, KF // 4)
            w2_tiles = []
            for d in range(ND):
                w2d = w2pool.tile([P, KF, P], BF16, tag="w2d", name="w2d")
                for fc in range(0, KF, W2_CHUNK):
                    nc.scalar.dma_start(w2d[:, fc:fc + W2_CHUNK],
                                        w2_d[d][:, fc:fc + W2_CHUNK])
                w2_tiles.append(w2d)

            # layer 2: yT[d] = sum_f W2[f,d].T @ hT[f] + b2[d]
            for d in range(ND):
                w2d = w2_tiles[d]
                for (g0, gn) in groups:
                    py = pyp.tile([P, 512], F32, tag="py", name="py")[:, :gn]
                    for f in range(KF):
                        nc.tensor.matmul(
                            py, w2d[:, f, :], h_sb[:, f, g0:g0 + gn],
                            start=(f == 0), stop=(f == KF - 1),
                        )
                    ysb = ypool.tile([P, 512], F32, tag="ysb", name="ysb")[:, :gn]
                    nc.vector.tensor_scalar_add(ysb, py, b2_sb[:, d:d + 1])
                    nc.sync.dma_start(yt_d[d][:, g0:g0 + gn], ysb)

    nc.compile()
    return nc


def _plan_groups(T_CORE: int) -> list[int]:
    """Split T_CORE into matmul token-group sizes <= 512 (PSUM bank limit),
    largest first: a fat first group gives the PE enough work per W1 chunk to
    outpace the self-clocked weight stream; the remainder runs last."""
    sizes = []
    left = T_CORE
    while left > 0:
        gn = min(512, left)
        sizes.append(gn)
        left -= gn
    return sizes


def kernel(x, Wr, br, W1, b1, W2, b2, top_k):
    x = np.asarray(x, dtype=np.float32)
    Wr = np.asarray(Wr, dtype=np.float32)
    br = np.asarray(br, dtype=np.float32)
    W1 = np.asarray(W1, dtype=np.float32)
    b1 = np.asarray(b1, dtype=np.float32)
    W2 = np.asarray(W2, dtype=np.float32)
    b2 = np.asarray(b2, dtype=np.float32)
    K = int(np.asarray(top_k))

    B, S, D = x.shape
    E = Wr.shape[1]
    F = W1.shape[2]
    T = B * S
    xt = x.reshape(T, D)

    # --- host router (replicated): f32 probs to match the reference, f64 top-k
    logits = xt @ Wr + br
    lmax = logits.max(axis=1, keepdims=True)
    pexp = np.exp(logits - lmax)
    probs = pexp / pexp.sum(axis=1, keepdims=True)          # [T, E] f32
    logits64 = xt.astype(np.float64) @ Wr.astype(np.float64) + br
    # top-k by descending prob, ties -> lower index (jax.lax.top_k semantics)
    topi = np.argsort(-logits64, axis=1, kind="stable")[:, :K]  # [T, K]

    # --- dispatch: token lists per expert, chunked into work items so one
    # SPMD wave (8 items, one per core) always fits in SBUF.  The harness
    # shapes route <= ~1100 tokens per expert, so this is a single wave.
    CAP = 1280
    tok_idx = [np.where((topi == e).any(axis=1))[0] for e in range(E)]
    items = []  # (expert, token-index-array)
    for e in range(E):
        ix = tok_idx[e]
        for s in range(0, max(len(ix), 1), CAP):
            items.append((e, ix[s:s + CAP]))
    waves = [items[i:i + N_CORES] for i in range(0, len(items), N_CORES)]

    max_c = max(len(ix) for _, ix in items)
    # Tokens sit on the matmul free axis everywhere, so T_CORE needs no
    # 128-alignment — pad the heaviest item's count to 8 (16B cachelines).
    T_CORE = max(8, int(np.ceil(max_c / 8)) * 8)
    groups = []
    off = 0
    for gn in _plan_groups(T_CORE):
        groups.append((off, gn))
        off += gn

    def expert_inputs(e, ix):
        xe = np.zeros((T_CORE, D), dtype=np.float32)
        xe[: len(ix)] = xt[ix]
        xte = np.ascontiguousarray(xe.T).astype(ml_dtypes.bfloat16)
        w1e = np.ascontiguousarray(
            W1[e].reshape(D // P, P, F // P, P).transpose(2, 1, 0, 3)
        ).astype(ml_dtypes.bfloat16)
        w2e = np.ascontiguousarray(
            W2[e].reshape(F // P, P, D // P, P).transpose(2, 1, 0, 3)
        ).astype(ml_dtypes.bfloat16)
        b1e = np.ascontiguousarray(b1[e].reshape(F // P, P).T)
        b2e = np.ascontiguousarray(b2[e].reshape(D // P, P).T)
        return {
            "xt": xte.reshape(D // P, P, T_CORE),
            "w1": w1e,
            "w2": w2e,
            "b1": b1e,
            "b2": b2e,
        }

    nc = _build_moe_ffn(T_CORE, groups, D, F)
    out = np.zeros((T, D), dtype=np.float32)
    global LAST_RESULTS
    for wave in waves:
        in_maps = [expert_inputs(e, ix) for e, ix in wave]
        while len(in_maps) < N_CORES:          # idle cores rerun item 0
            in_maps.append(in_maps[0])
        res = run_bass_kernel_spmd(nc, in_maps, core_ids=list(range(N_CORES)))
        LAST_RESULTS = res
        # combine: out[t] += probs[t, e] * y_e[slot(t)]
        for (e, ix), r in zip(wave, res.results):
            if len(ix) == 0:
                continue
            yte = r["yt"]                                  # [D//P, P, T_CORE]
            ye = yte.reshape(D, T_CORE)[:, : len(ix)].T    # [c, D] token rows
            out[ix] += probs[ix, e][:, None] * ye

    return out.reshape(B, S, D), probs.reshape(B, S, E)


# revision 18
# speedup vs baseline: 1.1635x; 1.0018x over previous
"""MoE layer (top-2 of 8 experts) on 8 Trainium2 NeuronCores.

Strategy (expert parallelism, host-side dispatch):
  - Router (x @ Wr -> softmax -> top-k) is computed on host: it is ~0.05% of
    the total FLOPs.  Decisions use float64 so near-ties resolve exactly.
  - Tokens are gathered per expert on host ("all-to-all dispatch"), padded to
    a common per-core count T_CORE, and each core runs its expert's FFN:
        hT = relu(W1e.T @ xT + b1e);  yT = W2e.T @ hT + b2e
    in bf16 on the tensor engine (fp32 PSUM accumulation).
  - Host applies the top-k combine weights and scatter-adds back ("combine").

Per-core device layout (everything 128-partition tiled):
  xt  [8,128,T]   bf16  x gathered+transposed, D on partitions (8 k-tiles)
  w1  [32,128,8,128] bf16  W1e chunk [f,p,k,:] = W1e[k*128+p, f*128:(f+1)*128]
  w2  [8,128,32,128] bf16  W2e chunk [d,p,f,:] = W2e[f*128+p, d*128:(d+1)*128]
  b1  [128,32] f32 (per-partition bias per f-chunk), b2 [128,8] f32
  yt  [8,128,T]   f32   output, D on partitions
"""

import numpy as np
import ml_dtypes

import concourse.mybir as mybir
import concourse.tile as tile
from concourse import bacc
from concourse.bass_utils import run_bass_kernel_spmd

BF16 = mybir.dt.bfloat16
F32 = mybir.dt.float32

N_CORES = 8
P = 128

# Populated by kernel() with the BassKernelResults of the device run so a
# test harness can read exec_time_ns when tracing is enabled (BASS_TRACE=1).
LAST_RESULTS = None


def _build_moe_ffn(T_CORE: int, groups: list[tuple[int, int]], D: int, F: int):
    """One expert's FFN over T_CORE tokens: yT = W2.T @ relu(W1.T @ xT + b1) + b2."""
    KD = D // P   # k-tiles over D (contraction of layer 1)
    KF = F // P   # f-chunks over F (rows of hT / contraction of layer 2)
    ND = D // P   # d-chunks of the output

    nc = bacc.Bacc("TRN2", target_bir_lowering=False, debug=False,
                   num_devices=N_CORES)
    xt_d = nc.dram_tensor("xt", [KD, P, T_CORE], BF16, kind="ExternalInput")
    w1_d = nc.dram_tensor("w1", [KF, P, KD, P], BF16, kind="ExternalInput")
    w2_d = nc.dram_tensor("w2", [ND, P, KF, P], BF16, kind="ExternalInput")
    b1_d = nc.dram_tensor("b1", [P, KF], F32, kind="ExternalInput")
    b2_d = nc.dram_tensor("b2", [P, ND], F32, kind="ExternalInput")
    yt_d = nc.dram_tensor("yt", [ND, P, T_CORE], F32, kind="ExternalOutput")

    # DMA placement notes: nc.sync/nc.scalar dma_start lowers to DMA_DIRECT2D
    # which OCCUPIES the issuing engine for the transfer duration, and each
    # engine's queue is FIFO.  So: activations + outputs ride the (otherwise
    # idle) sync queue; W1/W2 ride the scalar queue, with W1 DMAs interleaved
    # behind layer-1 ACTIVATEs so the weight stream is self-clocked by compute
    # progress and never blocks PSUM evacuation.
    W1_AHEAD = 8  # W1 f-chunks prefetched before the first matmul

    with tile.TileContext(nc) as tc:
        with (
            tc.tile_pool(name="resident", bufs=1) as rpool,
            tc.tile_pool(name="w2s", bufs=3) as w2pool,
            tc.tile_pool(name="yout", bufs=3) as ypool,
            tc.tile_pool(name="ph", bufs=5, space="PSUM") as php,
            tc.tile_pool(name="py", bufs=3, space="PSUM") as pyp,
        ):
            # sync queue: activations in group order (group 0's matmuls
            # only wait on group 0's slices).  Biases ride the scalar queue.
            b1_sb = rpool.tile([P, KF], F32, tag="b1")
            nc.scalar.dma_start(b1_sb[:], b1_d[:])
            b2_sb = rpool.tile([P, ND], F32, tag="b2")
            nc.scalar.dma_start(b2_sb[:], b2_d[:])
            xt_sb = rpool.tile([P, KD, T_CORE], BF16, tag="xt")
            for (g0, gn) in groups:
                for k in range(KD):
                    nc.sync.dma_start(xt_sb[:, k, g0:g0 + gn],
                                      xt_d[k][:, g0:g0 + gn])
            # scalar queue: initial W1 window; the rest is emitted inside the
            # first group's f-loop.
            w1_sb = rpool.tile([P, KF, KD, P], BF16, tag="w1")
            for f in range(min(W1_AHEAD, KF)):
                nc.scalar.dma_start(w1_sb[:, f], w1_d[f])
            h_sb = rpool.tile([P, KF, T_CORE], BF16, tag="h")

            # layer 1: hT[f] = relu(sum_k W1[k,f].T @ xT[k] + b1[f])
            # Group-outer: group g's PE work depends only on group g's
            # activation slices plus the self-clocked W1 stream.
            for gi, (g0, gn) in enumerate(groups):
                for f in range(KF):
                    ph = php.tile([P, 512], F32, tag="ph", name="ph")[:, :gn]
                    for k in range(KD):
                        nc.tensor.matmul(
                            ph, w1_sb[:, f, k, :], xt_sb[:, k, g0:g0 + gn],
                            start=(k == 0), stop=(k == KD - 1),
                        )
                    nc.scalar.activation(
                        h_sb[:, f, g0:g0 + gn], ph,
                        mybir.ActivationFunctionType.Relu,
                        bias=b1_sb[:, f:f + 1],
                    )
                    if gi == 0 and f + W1_AHEAD < KF:
                        nc.scalar.dma_start(w1_sb[:, f + W1_AHEAD],
                                            w1_d[f + W1_AHEAD])

            # W2 stream (scalar queue, chunked so layer 2's first matmuls only
            # wait on the first quarter of W2[0]).  With bufs=3 the 4th DMA
            # blocks the scalar queue until layer 2 frees a slot — harmless,
            # the scalar engine has no further work.
            W2_CHUNK = max(1, KF // 4)
            w2_tiles = []
            for d in range(ND):
                w2d = w2pool.tile([P, KF, P], BF16, tag="w2d", name="w2d")
                for fc in range(0, KF, W2_CHUNK):
                    nc.scalar.dma_start(w2d[:, fc:fc + W2_CHUNK],
                                        w2_d[d][:, fc:fc + W2_CHUNK])
                w2_tiles.append(w2d)

            # layer 2: yT[d] = sum_f W2[f,d].T @ hT[f] + b2[d]
            for d in range(ND):
                w2d = w2_tiles[d]
                for (g0, gn) in groups:
                    py = pyp.tile([P, 512], F32, tag="py", name="py")[:, :gn]
                    for f in range(KF):
                        nc.tensor.matmul(
                            py, w2d[:, f, :], h_sb[:, f, g0:g0 + gn],
                            start=(f == 0), stop=(f == KF - 1),
                        )
                    ysb = ypool.tile([P, 512], F32, tag="ysb", name="ysb")[:, :gn]
                    nc.vector.tensor_scalar_add(ysb, py, b2_sb[:, d:d + 1])
                    nc.sync.dma_start(yt_d[d][:, g0:g0 + gn], ysb)

    nc.compile()
    return nc


def _plan_groups(T_CORE: int) -> list[int]:
    """Split T_CORE into matmul token-group sizes <= 512 (PSUM bank limit),
    largest first: a fat first group gives the PE enough work per W1 chunk to
    outpace the self-clocked weight stream; the remainder runs last."""
    sizes = []
    left = T_CORE
    while left > 0:
        gn = min(512, left)
        sizes.append(gn)
        left -= gn
    return sizes


def kernel(x, Wr, br, W1, b1, W2, b2, top_k):
    x = np.asarray(x, dtype=np.float32)
    Wr = np.asarray(Wr, dtype=np.float32)
    br = np.asarray(br, dtype=np.float32)
    W1 = np.asarray(W1, dtype=np.float32)
    b1 = np.asarray(b1, dtype=np.float32)
    W2 = np.asarray(W2, dtype=np.float32)
    b2 = np.asarray(b2, dtype=np.float32)
    K = int(np.asarray(top_k))

    B, S, D = x.shape
    E = Wr.shape[1]
    F = W1.shape[2]
    T = B * S
    xt = x.reshape(T, D)

    # --- host router (replicated): f32 probs to match the reference, f64 top-k
    logits = xt @ Wr + br
    lmax = logits.max(axis=1, keepdims=True)
    pexp = np.exp(logits - lmax)
    probs = pexp / pexp.sum(axis=1, keepdims=True)          # [T, E] f32
    logits64 = xt.astype(np.float64) @ Wr.astype(np.float64) + br
    # top-k by descending prob, ties -> lower index (jax.lax.top_k semantics)
    topi = np.argsort(-logits64, axis=1, kind="stable")[:, :K]  # [T, K]

    # --- dispatch: token lists per expert, chunked into work items so one
    # SPMD wave (8 items, one per core) always fits in SBUF.  The harness
    # shapes route <= ~1100 tokens per expert, so this is a single wave.
    CAP = 1280
    tok_idx = [np.where((topi == e).any(axis=1))[0] for e in range(E)]
    items = []  # (expert, token-index-array)
    for e in range(E):
        ix = tok_idx[e]
        for s in range(0, max(len(ix), 1), CAP):
            items.append((e, ix[s:s + CAP]))
    waves = [items[i:i + N_CORES] for i in range(0, len(items), N_CORES)]

    max_c = max(len(ix) for _, ix in items)
    # Tokens sit on the matmul free axis everywhere, so T_CORE needs no
    # 128-alignment — pad the heaviest item's count to 8 (16B cachelines).
    T_CORE = max(8, int(np.ceil(max_c / 8)) * 8)
    groups = []
    off = 0
    for gn in _plan_groups(T_CORE):
        groups.append((off, gn))
        off += gn

    def expert_inputs(e, ix):
        xe = np.zeros((T_CORE, D), dtype=np.float32)
        xe[: len(ix)] = xt[ix]
        xte = np.ascontiguousarray(xe.T).astype(ml_dtypes.bfloat16)
        w1e = np.ascontiguousarray(
            W1[e].reshape(D // P, P, F // P, P).transpose(2, 1, 0, 3)
        ).astype(ml_dtypes.bfloat16)
        w2e = np.ascontiguousarray(
            W2[e].reshape(F // P, P, D // P, P).transpose(2, 1, 0, 3)
        ).astype(ml_dtypes.bfloat16)
        b1e = np.ascontiguousarray(b1[e].reshape(F // P, P).T)
        b2e = np.ascontiguousarray(b2[e].reshape(D // P, P).T)
        return {
            "xt": xte.reshape(D // P, P, T_CORE),
            "w1": w1e,
            "w2": w2e,
            "b1": b1e,
            "b2": b2e,
        }

    nc = _build_moe_ffn(T_CORE, groups, D, F)
    out = np.zeros((T, D), dtype=np.float32)
    global LAST_RESULTS
    for wave in waves:
        in_maps = [expert_inputs(e, ix) for e, ix in wave]
        while len(in_maps) < N_CORES:          # idle cores rerun item 0
            in_maps.append(in_maps[0])
        res = run_bass_kernel_spmd(nc, in_maps, core_ids=list(range(N_CORES)))
        LAST_RESULTS = res
        # combine: out[t] += probs[t, e] * y_e[slot(t)]
        for (e, ix), r in zip(wave, res.results):
            if len(ix) == 0:
                continue
            yte = r["yt"]                                  # [D//P, P, T_CORE]
            ye = yte.reshape(D, T_CORE)[:, : len(ix)].T    # [c, D] token rows
            out[ix] += probs[ix, e][:, None] * ye

    return out.reshape(B, S, D), probs.reshape(B, S, E)


# revision 19
# speedup vs baseline: 1.1660x; 1.0021x over previous
"""MoE layer (top-2 of 8 experts) on 8 Trainium2 NeuronCores.

Strategy (expert parallelism, host-side dispatch):
  - Router (x @ Wr -> softmax -> top-k) is computed on host: it is ~0.05% of
    the total FLOPs.  Decisions use float64 so near-ties resolve exactly.
  - Tokens are gathered per expert on host ("all-to-all dispatch"), padded to
    a common per-core count T_CORE, and each core runs its expert's FFN:
        hT = relu(W1e.T @ xT + b1e);  yT = W2e.T @ hT + b2e
    in bf16 on the tensor engine (fp32 PSUM accumulation).
  - Host applies the top-k combine weights and scatter-adds back ("combine").

Per-core device layout (everything 128-partition tiled):
  xt  [8,128,T]   bf16  x gathered+transposed, D on partitions (8 k-tiles)
  w1  [32,128,8,128] bf16  W1e chunk [f,p,k,:] = W1e[k*128+p, f*128:(f+1)*128]
  w2  [8,128,32,128] bf16  W2e chunk [d,p,f,:] = W2e[f*128+p, d*128:(d+1)*128]
  b1  [128,32] f32 (per-partition bias per f-chunk), b2 [128,8] f32
  yt  [8,128,T]   f32   output, D on partitions
"""

import numpy as np
import ml_dtypes

import concourse.mybir as mybir
import concourse.tile as tile
from concourse import bacc
from concourse.bass_utils import run_bass_kernel_spmd

BF16 = mybir.dt.bfloat16
F32 = mybir.dt.float32

N_CORES = 8
P = 128

# Populated by kernel() with the BassKernelResults of the device run so a
# test harness can read exec_time_ns when tracing is enabled (BASS_TRACE=1).
LAST_RESULTS = None


def _build_moe_ffn(T_CORE: int, groups: list[tuple[int, int]], D: int, F: int):
    """One expert's FFN over T_CORE tokens: yT = W2.T @ relu(W1.T @ xT + b1) + b2."""
    KD = D // P   # k-tiles over D (contraction of layer 1)
    KF = F // P   # f-chunks over F (rows of hT / contraction of layer 2)
    ND = D // P   # d-chunks of the output

    nc = bacc.Bacc("TRN2", target_bir_lowering=False, debug=False,
                   num_devices=N_CORES)
    xt_d = nc.dram_tensor("xt", [KD, P, T_CORE], BF16, kind="ExternalInput")
    w1_d = nc.dram_tensor("w1", [KF, P, KD, P], BF16, kind="ExternalInput")
    w2_d = nc.dram_tensor("w2", [ND, P, KF, P], BF16, kind="ExternalInput")
    b1_d = nc.dram_tensor("b1", [P, KF], F32, kind="ExternalInput")
    b2_d = nc.dram_tensor("b2", [P, ND], F32, kind="ExternalInput")
    yt_d = nc.dram_tensor("yt", [ND, P, T_CORE], BF16, kind="ExternalOutput")

    # DMA placement notes: nc.sync/nc.scalar dma_start lowers to DMA_DIRECT2D
    # which OCCUPIES the issuing engine for the transfer duration, and each
    # engine's queue is FIFO.  So: activations + outputs ride the (otherwise
    # idle) sync queue; W1/W2 ride the scalar queue, with W1 DMAs interleaved
    # behind layer-1 ACTIVATEs so the weight stream is self-clocked by compute
    # progress and never blocks PSUM evacuation.
    W1_AHEAD = 8  # W1 f-chunks prefetched before the first matmul

    with tile.TileContext(nc) as tc:
        with (
            tc.tile_pool(name="resident", bufs=1) as rpool,
            tc.tile_pool(name="w2s", bufs=3) as w2pool,
            tc.tile_pool(name="yout", bufs=3) as ypool,
            tc.tile_pool(name="ph", bufs=5, space="PSUM") as php,
            tc.tile_pool(name="py", bufs=3, space="PSUM") as pyp,
        ):
            # sync queue: activations in group order (group 0's matmuls
            # only wait on group 0's slices).  Biases ride the scalar queue.
            b1_sb = rpool.tile([P, KF], F32, tag="b1")
            nc.scalar.dma_start(b1_sb[:], b1_d[:])
            b2_sb = rpool.tile([P, ND], F32, tag="b2")
            nc.scalar.dma_start(b2_sb[:], b2_d[:])
            xt_sb = rpool.tile([P, KD, T_CORE], BF16, tag="xt")
            for (g0, gn) in groups:
                for k in range(KD):
                    nc.sync.dma_start(xt_sb[:, k, g0:g0 + gn],
                                      xt_d[k][:, g0:g0 + gn])
            # scalar queue: initial W1 window; the rest is emitted inside the
            # first group's f-loop.
            w1_sb = rpool.tile([P, KF, KD, P], BF16, tag="w1")
            for f in range(min(W1_AHEAD, KF)):
                nc.scalar.dma_start(w1_sb[:, f], w1_d[f])
            h_sb = rpool.tile([P, KF, T_CORE], BF16, tag="h")

            # layer 1: hT[f] = relu(sum_k W1[k,f].T @ xT[k] + b1[f])
            # Group-outer: group g's PE work depends only on group g's
            # activation slices plus the self-clocked W1 stream.
            for gi, (g0, gn) in enumerate(groups):
                for f in range(KF):
                    ph = php.tile([P, 512], F32, tag="ph", name="ph")[:, :gn]
                    for k in range(KD):
                        nc.tensor.matmul(
                            ph, w1_sb[:, f, k, :], xt_sb[:, k, g0:g0 + gn],
                            start=(k == 0), stop=(k == KD - 1),
                        )
                    nc.scalar.activation(
                        h_sb[:, f, g0:g0 + gn], ph,
                        mybir.ActivationFunctionType.Relu,
                        bias=b1_sb[:, f:f + 1],
                    )
                    if gi == 0 and f + W1_AHEAD < KF:
                        nc.scalar.dma_start(w1_sb[:, f + W1_AHEAD],
                                            w1_d[f + W1_AHEAD])

            # W2 stream (scalar queue, chunked so layer 2's first matmuls only
            # wait on the first quarter of W2[0]).  With bufs=3 the 4th DMA
            # blocks the scalar queue until layer 2 frees a slot — harmless,
            # the scalar engine has no further work.
            W2_CHUNK = max(1, KF // 4)
            w2_tiles = []
            for d in range(ND):
                w2d = w2pool.tile([P, KF, P], BF16, tag="w2d", name="w2d")
                for fc in range(0, KF, W2_CHUNK):
                    nc.scalar.dma_start(w2d[:, fc:fc + W2_CHUNK],
                                        w2_d[d][:, fc:fc + W2_CHUNK])
                w2_tiles.append(w2d)

            # layer 2: yT[d] = sum_f W2[f,d].T @ hT[f] + b2[d]
            for d in range(ND):
                w2d = w2_tiles[d]
                for (g0, gn) in groups:
                    py = pyp.tile([P, 512], F32, tag="py", name="py")[:, :gn]
                    for f in range(KF):
                        nc.tensor.matmul(
                            py, w2d[:, f, :], h_sb[:, f, g0:g0 + gn],
                            start=(f == 0), stop=(f == KF - 1),
                        )
                    ysb = ypool.tile([P, 512], BF16, tag="ysb", name="ysb")[:, :gn]
                    nc.vector.tensor_scalar_add(ysb, py, b2_sb[:, d:d + 1])
                    nc.sync.dma_start(yt_d[d][:, g0:g0 + gn], ysb)

    nc.compile()
    return nc


def _plan_groups(T_CORE: int) -> list[int]:
    """Split T_CORE into matmul token-group sizes <= 512 (PSUM bank limit),
    largest first: a fat first group gives the PE enough work per W1 chunk to
    outpace the self-clocked weight stream; the remainder runs last."""
    sizes = []
    left = T_CORE
    while left > 0:
        gn = min(512, left)
        sizes.append(gn)
        left -= gn
    return sizes


def kernel(x, Wr, br, W1, b1, W2, b2, top_k):
    x = np.asarray(x, dtype=np.float32)
    Wr = np.asarray(Wr, dtype=np.float32)
    br = np.asarray(br, dtype=np.float32)
    W1 = np.asarray(W1, dtype=np.float32)
    b1 = np.asarray(b1, dtype=np.float32)
    W2 = np.asarray(W2, dtype=np.float32)
    b2 = np.asarray(b2, dtype=np.float32)
    K = int(np.asarray(top_k))

    B, S, D = x.shape
    E = Wr.shape[1]
    F = W1.shape[2]
    T = B * S
    xt = x.reshape(T, D)

    # --- host router (replicated): f32 probs to match the reference, f64 top-k
    logits = xt @ Wr + br
    lmax = logits.max(axis=1, keepdims=True)
    pexp = np.exp(logits - lmax)
    probs = pexp / pexp.sum(axis=1, keepdims=True)          # [T, E] f32
    logits64 = xt.astype(np.float64) @ Wr.astype(np.float64) + br
    # top-k by descending prob, ties -> lower index (jax.lax.top_k semantics)
    topi = np.argsort(-logits64, axis=1, kind="stable")[:, :K]  # [T, K]

    # --- dispatch: token lists per expert, chunked into work items so one
    # SPMD wave (8 items, one per core) always fits in SBUF.  The harness
    # shapes route <= ~1100 tokens per expert, so this is a single wave.
    CAP = 1280
    tok_idx = [np.where((topi == e).any(axis=1))[0] for e in range(E)]
    items = []  # (expert, token-index-array)
    for e in range(E):
        ix = tok_idx[e]
        for s in range(0, max(len(ix), 1), CAP):
            items.append((e, ix[s:s + CAP]))
    waves = [items[i:i + N_CORES] for i in range(0, len(items), N_CORES)]

    max_c = max(len(ix) for _, ix in items)
    # Tokens sit on the matmul free axis everywhere, so T_CORE needs no
    # 128-alignment — pad the heaviest item's count to 8 (16B cachelines).
    T_CORE = max(8, int(np.ceil(max_c / 8)) * 8)
    groups = []
    off = 0
    for gn in _plan_groups(T_CORE):
        groups.append((off, gn))
        off += gn

    def expert_inputs(e, ix):
        xe = np.zeros((T_CORE, D), dtype=np.float32)
        xe[: len(ix)] = xt[ix]
        xte = np.ascontiguousarray(xe.T).astype(ml_dtypes.bfloat16)
        w1e = np.ascontiguousarray(
            W1[e].reshape(D // P, P, F // P, P).transpose(2, 1, 0, 3)
        ).astype(ml_dtypes.bfloat16)
        w2e = np.ascontiguousarray(
            W2[e].reshape(F // P, P, D // P, P).transpose(2, 1, 0, 3)
        ).astype(ml_dtypes.bfloat16)
        b1e = np.ascontiguousarray(b1[e].reshape(F // P, P).T)
        b2e = np.ascontiguousarray(b2[e].reshape(D // P, P).T)
        return {
            "xt": xte.reshape(D // P, P, T_CORE),
            "w1": w1e,
            "w2": w2e,
            "b1": b1e,
            "b2": b2e,
        }

    nc = _build_moe_ffn(T_CORE, groups, D, F)
    out = np.zeros((T, D), dtype=np.float32)
    global LAST_RESULTS
    for wave in waves:
        in_maps = [expert_inputs(e, ix) for e, ix in wave]
        while len(in_maps) < N_CORES:          # idle cores rerun item 0
            in_maps.append(in_maps[0])
        res = run_bass_kernel_spmd(nc, in_maps, core_ids=list(range(N_CORES)))
        LAST_RESULTS = res
        # combine: out[t] += probs[t, e] * y_e[slot(t)]
        for (e, ix), r in zip(wave, res.results):
            if len(ix) == 0:
                continue
            yte = r["yt"].astype(np.float32)               # [D//P, P, T_CORE]
            ye = yte.reshape(D, T_CORE)[:, : len(ix)].T    # [c, D] token rows
            out[ix] += probs[ix, e][:, None] * ye

    return out.reshape(B, S, D), probs.reshape(B, S, E)
